# revision 1
# baseline (speedup 1.0000x reference)
"""Grok1-style attention on 8 trn2 NeuronCores, tensor-parallel over heads.

Sharding (per core c of 8):
  - q heads 4c..4c+3 (512 features), kv head c (128+128 features)
  - w_qkv sharded column-wise (by head), w_o row-wise; partial o_proj
    outputs summed on host (the all-reduce).

Device layout trick: qkv is computed TRANSPOSED (features on partitions,
positions on free axis), so scores (k^T q), probs*V and o_proj chain with
no transposes except 16 cheap PE transposes of V.

Softmax: tanh logit-cap bounds scores to +-30 so exp() cannot overflow ->
no row-max pass. Row sums via ones-vector matmul on the PE (scores are
held k-on-partitions); normalization via a rank-1 broadcast matmul.

Precision: bf16 matmul operands (PE runs fp32 4x slower), fp32 PSUM
accumulation, tanh kept in fp32 (bf16 there would put ~0.06 absolute
error into the exponent).
"""
import numpy as np
from contextlib import ExitStack

import concourse.bass as bass
import concourse.mybir as mybir
import concourse.tile as tile
from concourse import bacc
from concourse.bass_utils import run_bass_kernel_spmd
from concourse.masks import make_identity

T = 2048
D = 4096
HD = 128
HALF = 64
NCORES = 8
HPC = 4                    # q heads per core
QF = HPC * HD              # 512
NF = QF + 2 * HD           # 768 qkv features per core
NCH = D // 128             # 32 contraction chunks
TT = 512                   # t-tile width (matmul moving dim)
NTT = T // TT              # 4
NKT = T // 128             # 16 k-tiles
SCALING = HD ** -0.5
CAP = 30.0
BF = mybir.dt.bfloat16
F32 = mybir.dt.float32


def _emit(nc):
    hT = nc.dram_tensor("hT", [D, T], F32, kind="ExternalInput").ap()
    wq = nc.dram_tensor("wq", [D, NF], F32, kind="ExternalInput").ap()
    wo = nc.dram_tensor("wo", [QF, D], F32, kind="ExternalInput").ap()
    cc = nc.dram_tensor("cc", [HD, T], F32, kind="ExternalInput").ap()
    ss = nc.dram_tensor("ss", [HD, T], F32, kind="ExternalInput").ap()
    mk = nc.dram_tensor("mk", [4, 128, TT], F32, kind="ExternalInput").ap()
    out = nc.dram_tensor("out", [T, D], F32, kind="ExternalOutput").ap()

    with tile.TileContext(nc) as tc:
        with ExitStack() as ctx:
            wqp = ctx.enter_context(tc.tile_pool(name="wqp", bufs=1))
            bigp = ctx.enter_context(tc.tile_pool(name="bigp", bufs=2))
            cstp = ctx.enter_context(tc.tile_pool(name="cstp", bufs=1))
            seqp = ctx.enter_context(tc.tile_pool(name="seqp", bufs=1))
            rtp = ctx.enter_context(tc.tile_pool(name="rtp", bufs=3))
            stp = ctx.enter_context(tc.tile_pool(name="stp", bufs=4))
            etp = ctx.enter_context(tc.tile_pool(name="etp", bufs=8))
            smp = ctx.enter_context(tc.tile_pool(name="smp", bufs=2))
            obp = ctx.enter_context(tc.tile_pool(name="obp", bufs=8))
            psp = ctx.enter_context(tc.tile_pool(name="psp", bufs=1, space="PSUM"))

            # ---- constants + resident loads ----
            wq_sb = wqp.tile([128, NCH, NF], BF, tag="wq")
            nc.gpsimd.dma_start(wq_sb[:], wq.rearrange("(c p) f -> p c f", p=128))
            cc_sb = cstp.tile([HD, T], BF, tag="cc")
            ss_sb = cstp.tile([HD, T], BF, tag="ss")
            nc.gpsimd.dma_start(cc_sb[:], cc[:, :])
            nc.gpsimd.dma_start(ss_sb[:], ss[:, :])
            ident = cstp.tile([128, 128], BF, tag="id")
            make_identity(nc, ident[:])
            ones_k = cstp.tile([128, 1], BF, tag="ones_k")
            nc.gpsimd.memset(ones_k[:], 1.0)
            ones_m = cstp.tile([1, 128], BF, tag="ones_m")
            nc.gpsimd.memset(ones_m[:], 1.0)
            mk_sb = cstp.tile([128, 4, TT], BF, tag="mk")
            nc.gpsimd.dma_start(mk_sb[:], mk.rearrange("m p t -> p m t"))

            # per-t-tile tiles so later phases can start early
            qTt = [[seqp.tile([HD, TT], BF, tag=f"q{h}_{tt}", name=f"qT{h}_{tt}")
                    for tt in range(NTT)] for h in range(HPC)]
            kTt = [seqp.tile([HD, TT], BF, tag=f"k_{tt}", name=f"kT{tt}")
                   for tt in range(NTT)]
            vTt = [seqp.tile([HD, TT], BF, tag=f"v_{tt}", name=f"vT{tt}")
                   for tt in range(NTT)]
            vbt = [seqp.tile([128, HD], BF, tag=f"vb_{kt}", name=f"vb{kt}")
                   for kt in range(NKT)]
            atq = [[seqp.tile([HD, TT], BF, tag=f"a{h}_{qt}", name=f"at{h}_{qt}")
                    for qt in range(NTT)] for h in range(HPC)]

            # ---- phase 1: qkv projection (transposed) + rope ----
            # psum banks: b0-b2 qkv accum, b3 transposes,
            #             b4-b5 scores, b6 attn accum, b7 denom+bcast
            hT_r = hT.rearrange("(c p) t -> p c t", p=128)
            for tt in range(NTT):
                t0 = tt * TT
                h_a = bigp.tile([128, NCH // 2, TT], BF, tag="big", name="h_a")
                nc.gpsimd.dma_start(h_a[:], hT_r[:, 0:NCH // 2, t0:t0 + TT])
                h_b = bigp.tile([128, NCH // 2, TT], BF, tag="big", name="h_b")
                nc.gpsimd.dma_start(h_b[:], hT_r[:, NCH // 2:NCH, t0:t0 + TT])
                c_t = cc_sb[:, t0:t0 + TT]
                s_t = ss_sb[:, t0:t0 + TT]
                for fg in range(2):
                    ps3 = [psp.tile([128, TT], F32, tag=f"b{j}", name=f"qkv_ps{j}")
                           for j in range(3)]
                    for c in range(NCH):
                        src = (h_a if c < NCH // 2 else h_b)[:, c % (NCH // 2), :]
                        for j in range(3):
                            f = fg * 3 + j
                            nc.tensor.matmul(
                                ps3[j][:],
                                wq_sb[:, c, f * 128:(f + 1) * 128],
                                src,
                                start=(c == 0),
                                stop=(c == NCH - 1),
                            )
                    for j in range(3):
                        f = fg * 3 + j
                        if f < 5:
                            dst = qTt[f][tt] if f < HPC else kTt[tt]
                            qk_sb = rtp.tile([128, TT], BF, tag="qk_sb")
                            nc.vector.tensor_copy(qk_sb[:], ps3[j][:])
                            # rotated copy: [x2; x1] via partition-swap DMA
                            rot = rtp.tile([128, TT], BF, tag="rot")
                            nc.sync.dma_start(rot[0:HALF, :], qk_sb[HALF:128, :])
                            nc.sync.dma_start(rot[HALF:128, :], qk_sb[0:HALF, :])
                            m1 = rtp.tile([128, TT], BF, tag="m1")
                            nc.vector.tensor_mul(m1[:], qk_sb[:], c_t)
                            m2 = rtp.tile([128, TT], BF, tag="m2")
                            nc.vector.tensor_mul(m2[:], rot[:], s_t)
                            nc.vector.tensor_add(dst[:], m1[:], m2[:])
                        else:
                            nc.vector.tensor_copy(vTt[tt][:], ps3[j][:])
                # transpose this t-tile's V to [t, d] blocks
                for i in range(4):
                    kt = 4 * tt + i
                    tp = psp.tile([128, 128], BF, tag="b3", name="tp")
                    nc.tensor.transpose(tp[:], vTt[tt][:, i * 128:(i + 1) * 128], ident[:])
                    nc.vector.tensor_copy(vbt[kt][:], tp[:])

            # ---- phase 2: attention, qt-major so o_proj unblocks early ----
            for qt in range(NTT):
                for h in range(HPC):
                    a_ps = psp.tile([HD, TT], F32, tag="b6", name="a_ps")
                    d_ps = psp.tile([1, TT], F32, tag="b7", name="d_ps")
                    nkt = 4 * qt + 4
                    for kt in range(nkt):
                        m = kt - 4 * qt
                        j0 = 128 * m if m >= 0 else 0  # skip fully-masked cols
                        nw = TT - j0
                        s_ps = psp.tile([128, TT], F32, tag=f"b{4 + kt % 2}", name="s_ps")
                        nc.tensor.matmul(
                            s_ps[:, j0:TT], kTt[kt // 4][:, (kt % 4) * 128:(kt % 4 + 1) * 128],
                            qTt[h][qt][:, j0:TT],
                            start=True, stop=True,
                        )
                        st = stp.tile([128, TT], F32, tag="st")
                        nc.scalar.activation(
                            st[:, j0:TT], s_ps[:, j0:TT],
                            mybir.ActivationFunctionType.Tanh,
                            scale=SCALING / CAP,
                        )
                        et = etp.tile([128, TT], BF, tag="et")
                        nc.scalar.activation(
                            et[:, j0:TT], st[:, j0:TT],
                            mybir.ActivationFunctionType.Exp,
                            scale=CAP,
                        )
                        if m >= 0:
                            # causal mask: zero where k0+i > q0+j
                            nc.vector.tensor_mul(
                                et[:, j0:TT], et[:, j0:TT], mk_sb[:, m, j0:TT])
                        last = kt == nkt - 1
                        nc.tensor.matmul(
                            a_ps[:, j0:TT], vbt[kt][:], et[:, j0:TT],
                            start=(kt == 0), stop=last,
                        )
                        nc.tensor.matmul(
                            d_ps[:, j0:TT], ones_k[:], et[:, j0:TT],
                            start=(kt == 0), stop=last,
                        )
                    rc = smp.tile([1, TT], F32, tag="rc")
                    nc.vector.reciprocal(rc[:], d_ps[:])
                    rcb = smp.tile([1, TT], BF, tag="rcb")
                    nc.vector.tensor_copy(rcb[:], rc[:])
                    bc_ps = psp.tile([128, TT], F32, tag="b7", name="bc_ps")
                    nc.tensor.matmul(bc_ps[:], ones_m[:], rcb[:], start=True, stop=True)
                    bc_sb = smp.tile([128, TT], F32, tag="bcs")
                    nc.vector.tensor_copy(bc_sb[:], bc_ps[:])
                    nc.vector.tensor_mul(atq[h][qt][:], a_ps[:], bc_sb[:])

            # ---- phase 3: o_proj partial (psum banks b0-b3) ----
            wo_r = wo.rearrange("(c p) n -> p c n", p=128)
            wo_t = []
            for j in range(2):
                w_j = bigp.tile([128, 2, D], BF, tag="big", name=f"wo{j}")
                nc.gpsimd.dma_start(w_j[:], wo_r[:, 2 * j:2 * j + 2, :])
                wo_t.append(w_j)
            for t16 in range(T // 128):
                t0 = t16 * 128
                for half in range(2):
                    pls = [psp.tile([128, TT], F32, tag=f"b{n}", name=f"o_ps{n}")
                           for n in range(4)]
                    for fc in range(HPC):
                        lhsT = atq[fc][t16 // 4][:, (t16 % 4) * 128:(t16 % 4 + 1) * 128]
                        for n in range(4):
                            n0 = (half * 4 + n) * TT
                            nc.tensor.matmul(
                                pls[n][:], lhsT, wo_t[fc // 2][:, fc % 2, n0:n0 + TT],
                                start=(fc == 0), stop=(fc == HPC - 1),
                            )
                    for n in range(4):
                        n0 = (half * 4 + n) * TT
                        ob = obp.tile([128, TT], F32, tag="ob")
                        nc.vector.tensor_copy(ob[:], pls[n][:])
                        nc.sync.dma_start(out[t0:t0 + 128, n0:n0 + TT], ob[:])
    return nc


_CACHE = {}


def _get_nc():
    if "nc" not in _CACHE:
        nc = bacc.Bacc("TRN2", target_bir_lowering=False, debug=False)
        _emit(nc)
        nc.compile()
        _CACHE["nc"] = nc
    return _CACHE["nc"]


def _in_maps(positions, hidden_states, w_qkv, w_o):
    hidden_states = np.asarray(hidden_states, dtype=np.float32)
    w_qkv = np.asarray(w_qkv, dtype=np.float32)
    w_o = np.asarray(w_o, dtype=np.float32)
    pos = np.asarray(positions).astype(np.float64)

    hT = np.ascontiguousarray(hidden_states.T)
    inv_freq = 1.0 / (10000.0 ** (np.arange(HALF, dtype=np.float64) * 2.0 / HD))
    ang = np.outer(inv_freq, pos)                      # [64, T]
    cos = np.cos(ang).astype(np.float32)
    sin = np.sin(ang).astype(np.float32)
    cc = np.ascontiguousarray(np.concatenate([cos, cos], axis=0))   # [128, T]
    ss = np.ascontiguousarray(np.concatenate([-sin, sin], axis=0))  # [128, T]
    ii = np.arange(128)[:, None]
    jj = np.arange(TT)[None, :]
    mk = np.stack([(jj - ii - 128 * m >= 0) for m in range(4)]).astype(np.float32)

    in_maps = []
    for c in range(NCORES):
        rows = np.concatenate([
            w_qkv[QF * c:QF * (c + 1)],
            w_qkv[D + HD * c:D + HD * (c + 1)],
            w_qkv[D + HD * NCORES + HD * c:D + HD * NCORES + HD * (c + 1)],
        ], axis=0)                                      # [768, 4096]
        wq_c = np.ascontiguousarray(rows.T)             # [4096, 768]
        wo_c = np.ascontiguousarray(w_o[:, QF * c:QF * (c + 1)].T)  # [512, 4096]
        in_maps.append({"hT": hT, "wq": wq_c, "wo": wo_c, "cc": cc, "ss": ss, "mk": mk})
    return in_maps


def run(positions, hidden_states, w_qkv, w_o, trace=False):
    nc = _get_nc()
    in_maps = _in_maps(positions, hidden_states, w_qkv, w_o)
    res = run_bass_kernel_spmd(nc, in_maps, list(range(NCORES)), trace=trace)
    parts = np.stack([res.results[i]["out"] for i in range(NCORES)], axis=0)
    full = parts.sum(axis=0, dtype=np.float64).astype(np.float32)
    return full, res


def kernel(positions, hidden_states, w_qkv, w_o):
    full, _ = run(positions, hidden_states, w_qkv, w_o, trace=False)
    return full



# revision 3
# speedup vs baseline: 1.0716x; 1.0716x over previous
"""Grok1-style attention on 8 trn2 NeuronCores, tensor-parallel over heads.

Sharding (per core c of 8):
  - q heads 4c..4c+3 (512 features), kv head c (128+128 features)
  - w_qkv sharded column-wise (by head), w_o row-wise; partial o_proj
    outputs summed on host (the all-reduce).

Device layout: qkv computed TRANSPOSED (features on partitions, positions
on free axis) so scores (k^T q), probs*V and o_proj chain without
transposes except 16 cheap PE transposes of V.

Perf structure vs the naive version:
  - all inputs host-cast to bf16 and split across 3 DMA rings (gpsimd /
    sync / vector) so the first qkv matmul starts ~3us in, not 70us.
  - softmax denominator via an ALL-ONES [128,128] lhsT: every PSUM
    partition gets the row sum, so reciprocal runs 128-lane wide and the
    broadcast matmul + copy disappear.
  - 4-bank score runway (P0-P3) + o_proj(qt) interleaved after each qt's
    attention keeps PE fed while ACT (tanh+exp) is the pacer.
  - causal mask multiply on the Pool engine (et is SBUF-resident).
  - bf16 partial outputs; host sums in fp64.
"""
import numpy as np
from contextlib import ExitStack

import concourse.bass as bass
import concourse.mybir as mybir
import concourse.tile as tile
from concourse import bacc
from concourse.bass_utils import run_bass_kernel_spmd
from concourse.masks import make_identity

T = 2048
D = 4096
HD = 128
HALF = 64
NCORES = 8
HPC = 4                    # q heads per core
QF = HPC * HD              # 512
NF = QF + 2 * HD           # 768 qkv features per core
NCH = D // 128             # 32 contraction chunks
TT = 512                   # t-tile width (matmul moving dim)
NTT = T // TT              # 4
NKT = T // 128             # 16 k-tiles
SCALING = HD ** -0.5
CAP = 30.0
BF = mybir.dt.bfloat16
F32 = mybir.dt.float32
BF_NP = mybir.dt.np(BF)


def _emit(nc):
    hT = nc.dram_tensor("hT", [D, T], BF, kind="ExternalInput").ap()
    wq = nc.dram_tensor("wq", [D, NF], BF, kind="ExternalInput").ap()
    wo = nc.dram_tensor("wo", [QF, D], BF, kind="ExternalInput").ap()
    cc = nc.dram_tensor("cc", [HD, T], BF, kind="ExternalInput").ap()
    ss = nc.dram_tensor("ss", [HD, T], BF, kind="ExternalInput").ap()
    mk = nc.dram_tensor("mk", [4, 128, TT], BF, kind="ExternalInput").ap()
    out = nc.dram_tensor("out", [T, D], BF, kind="ExternalOutput").ap()

    with tile.TileContext(nc) as tc:
        with ExitStack() as ctx:
            wqp = ctx.enter_context(tc.tile_pool(name="wqp", bufs=1))
            hp = ctx.enter_context(tc.tile_pool(name="hp", bufs=4))
            cstp = ctx.enter_context(tc.tile_pool(name="cstp", bufs=1))
            seqp = ctx.enter_context(tc.tile_pool(name="seqp", bufs=1))
            rtp = ctx.enter_context(tc.tile_pool(name="rtp", bufs=2))
            stp = ctx.enter_context(tc.tile_pool(name="stp", bufs=3))
            etp = ctx.enter_context(tc.tile_pool(name="etp", bufs=8))
            smp = ctx.enter_context(tc.tile_pool(name="smp", bufs=2))
            obp = ctx.enter_context(tc.tile_pool(name="obp", bufs=6))
            psp = ctx.enter_context(tc.tile_pool(name="psp", bufs=1, space="PSUM"))

            # ---- resident loads, split across DMA rings ----
            # gpsimd ring: wq (8 chunks) then wo; sync ring: h tiles (and
            # later the out stores); vector ring: cc/ss/mk.
            wq_r = wq.rearrange("(c p) f -> p c f", p=128)
            wq_t = []
            for g in range(8):
                w_g = wqp.tile([128, 4, NF], BF, tag=f"wq{g}", name=f"wq{g}")
                nc.gpsimd.dma_start(w_g[:], wq_r[:, 4 * g:4 * g + 4, :])
                wq_t.append(w_g)
            cc_sb = cstp.tile([HD, T], BF, tag="cc")
            ss_sb = cstp.tile([HD, T], BF, tag="ss")
            nc.scalar.dma_start(cc_sb[:], cc[:, :])
            nc.scalar.dma_start(ss_sb[:], ss[:, :])
            mk_sb = cstp.tile([128, 4, TT], BF, tag="mk")
            nc.scalar.dma_start(mk_sb[:], mk.rearrange("m p t -> p m t"))
            wo_r = wo.rearrange("(c p) n -> p c n", p=128)
            wo_t = []
            for j in range(2):
                w_j = wqp.tile([128, 2, D], BF, tag=f"wo{j}", name=f"wo{j}")
                nc.gpsimd.dma_start(w_j[:], wo_r[:, 2 * j:2 * j + 2, :])
                wo_t.append(w_j)
            ident = cstp.tile([128, 128], BF, tag="id")
            make_identity(nc, ident[:])
            ones_kk = cstp.tile([128, 128], BF, tag="ones_kk")
            nc.gpsimd.memset(ones_kk[:], 1.0)

            qTt = [[seqp.tile([HD, TT], BF, tag=f"q{h}_{tt}", name=f"qT{h}_{tt}")
                    for tt in range(NTT)] for h in range(HPC)]
            kTt = [seqp.tile([HD, TT], BF, tag=f"k_{tt}", name=f"kT{tt}")
                   for tt in range(NTT)]
            vTt = [seqp.tile([HD, TT], BF, tag=f"v_{tt}", name=f"vT{tt}")
                   for tt in range(NTT)]
            vbt = [seqp.tile([128, HD], BF, tag=f"vb_{kt}", name=f"vb{kt}")
                   for kt in range(NKT)]
            atq = [[seqp.tile([HD, TT], BF, tag=f"a{h}_{qt}", name=f"at{h}_{qt}")
                    for qt in range(NTT)] for h in range(HPC)]

            # ---- phase 1: qkv projection (transposed) + rope ----
            # PSUM tags: P0-P2 qkv accum (reused by scores later), P3
            # transposes (later 4th score bank), P4 attn accum, P5 denom,
            # P6-P7 o_proj.
            hT_r = hT.rearrange("(c p) t -> p c t", p=128)
            for tt in range(NTT):
                t0 = tt * TT
                h_t = []
                for i in range(4):
                    h_i = hp.tile([128, 8, TT], BF, tag="h", name=f"h{tt}_{i}")
                    nc.sync.dma_start(h_i[:], hT_r[:, 8 * i:8 * i + 8, t0:t0 + TT])
                    h_t.append(h_i)
                c_t = cc_sb[:, t0:t0 + TT]
                s_t = ss_sb[:, t0:t0 + TT]
                for fg in range(2):
                    ps3 = [psp.tile([128, TT], F32, tag=f"P{j}", name=f"qkv_ps{j}")
                           for j in range(3)]
                    for c in range(NCH):
                        src = h_t[c // 8][:, c % 8, :]
                        for j in range(3):
                            f = fg * 3 + j
                            nc.tensor.matmul(
                                ps3[j][:],
                                wq_t[c // 4][:, c % 4, f * 128:(f + 1) * 128],
                                src,
                                start=(c == 0),
                                stop=(c == NCH - 1),
                            )
                    for j in range(3):
                        f = fg * 3 + j
                        if f < 5:
                            dst = qTt[f][tt] if f < HPC else kTt[tt]
                            qk_sb = rtp.tile([128, TT], BF, tag="qk_sb")
                            nc.scalar.copy(qk_sb[:], ps3[j][:])
                            # rotated copy: [x2; x1] via partition-swap DMA
                            rot = rtp.tile([128, TT], BF, tag="rot")
                            nc.gpsimd.dma_start(rot[0:HALF, :], qk_sb[HALF:128, :])
                            nc.gpsimd.dma_start(rot[HALF:128, :], qk_sb[0:HALF, :])
                            m1 = rtp.tile([128, TT], BF, tag="m1")
                            nc.vector.tensor_mul(m1[:], qk_sb[:], c_t)
                            m2 = rtp.tile([128, TT], BF, tag="m2")
                            nc.vector.tensor_mul(m2[:], rot[:], s_t)
                            nc.vector.tensor_add(dst[:], m1[:], m2[:])
                        else:
                            nc.scalar.copy(vTt[tt][:], ps3[j][:])
                # transpose this t-tile's V to [t, d] blocks
                for i in range(4):
                    kt = 4 * tt + i
                    tp = psp.tile([128, 128], BF, tag="P3", name="tp")
                    nc.tensor.transpose(tp[:], vTt[tt][:, i * 128:(i + 1) * 128], ident[:])
                    nc.vector.tensor_copy(vbt[kt][:], tp[:])

            # ---- phase 2+3: attention with 4-deep score runway, o_proj
            #      for qt interleaved right after its 4 heads finish ----
            def emit_score(qt, h, kt):
                m = kt - 4 * qt
                j0 = 128 * m if m >= 0 else 0
                s_ps = psp.tile([128, TT], F32, tag=f"P{kt % 4}", name="s_ps")
                nc.tensor.matmul(
                    s_ps[:, j0:TT],
                    kTt[kt // 4][:, (kt % 4) * 128:(kt % 4 + 1) * 128],
                    qTt[h][qt][:, j0:TT],
                    start=True, stop=True,
                )
                return s_ps, j0, m

            for qt in range(NTT):
                for h in range(HPC):
                    a_ps = psp.tile([HD, TT], F32, tag="P4", name="a_ps")
                    d_ps = psp.tile([128, TT], F32, tag="P5", name="d_ps")
                    nkt = 4 * qt + 4
                    pend = [emit_score(qt, h, kt) for kt in range(min(4, nkt))]
                    for kt in range(nkt):
                        s_ps, j0, m = pend[kt]
                        st = stp.tile([128, TT], F32, tag="st")
                        nc.scalar.activation(
                            st[:, j0:TT], s_ps[:, j0:TT],
                            mybir.ActivationFunctionType.Tanh,
                            scale=SCALING / CAP,
                        )
                        et = etp.tile([128, TT], BF, tag="et")
                        nc.scalar.activation(
                            et[:, j0:TT], st[:, j0:TT],
                            mybir.ActivationFunctionType.Exp,
                            scale=CAP,
                        )
                        if m >= 0:
                            # causal mask: zero where k0+i > q0+j (Pool)
                            nc.gpsimd.tensor_mul(
                                et[:, j0:TT], et[:, j0:TT], mk_sb[:, m, j0:TT])
                        last = kt == nkt - 1
                        nc.tensor.matmul(
                            a_ps[:, j0:TT], vbt[kt][:], et[:, j0:TT],
                            start=(kt == 0), stop=last,
                        )
                        nc.tensor.matmul(
                            d_ps[:, j0:TT], ones_kk[:], et[:, j0:TT],
                            start=(kt == 0), stop=last,
                        )
                        if kt + 4 < nkt:
                            pend.append(emit_score(qt, h, kt + 4))
                    rcp = smp.tile([128, TT], F32, tag="rcp")
                    nc.vector.reciprocal(rcp[:], d_ps[:])
                    nc.vector.tensor_mul(atq[h][qt][:], a_ps[:], rcp[:])

                # o_proj for this qt's 512 rows (PSUM P6/P7 ping-pong)
                for t16 in range(4 * qt, 4 * qt + 4):
                    for half in range(2):
                        for n in range(4):
                            n0 = (half * 4 + n) * TT
                            pl = psp.tile([128, TT], F32, tag=f"P{6 + n % 2}",
                                          name="o_ps")
                            for fc in range(HPC):
                                lhsT = atq[fc][qt][:, (t16 % 4) * 128:(t16 % 4 + 1) * 128]
                                nc.tensor.matmul(
                                    pl[:], lhsT, wo_t[fc // 2][:, fc % 2, n0:n0 + TT],
                                    start=(fc == 0), stop=(fc == HPC - 1),
                                )
                            ob = obp.tile([128, TT], BF, tag="ob")
                            nc.vector.tensor_copy(ob[:], pl[:])
                            nc.sync.dma_start(
                                out[t16 * 128:t16 * 128 + 128, n0:n0 + TT], ob[:])
    return nc


_CACHE = {}


def _get_nc():
    if "nc" not in _CACHE:
        nc = bacc.Bacc("TRN2", target_bir_lowering=False, debug=False)
        _emit(nc)
        nc.compile()
        _CACHE["nc"] = nc
    return _CACHE["nc"]


def _in_maps(positions, hidden_states, w_qkv, w_o):
    hidden_states = np.asarray(hidden_states, dtype=np.float32)
    w_qkv = np.asarray(w_qkv, dtype=np.float32)
    w_o = np.asarray(w_o, dtype=np.float32)
    pos = np.asarray(positions).astype(np.float64)

    hT = np.ascontiguousarray(hidden_states.T).astype(BF_NP)
    inv_freq = 1.0 / (10000.0 ** (np.arange(HALF, dtype=np.float64) * 2.0 / HD))
    ang = np.outer(inv_freq, pos)                      # [64, T]
    cos = np.cos(ang).astype(np.float32)
    sin = np.sin(ang).astype(np.float32)
    ccm = np.concatenate([cos, cos], axis=0).astype(BF_NP)   # [128, T]
    ssm = np.concatenate([-sin, sin], axis=0).astype(BF_NP)  # [128, T]
    ii = np.arange(128)[:, None]
    jj = np.arange(TT)[None, :]
    mkm = np.stack([(jj - ii - 128 * m >= 0) for m in range(4)]).astype(BF_NP)

    in_maps = []
    for c in range(NCORES):
        rows = np.concatenate([
            w_qkv[QF * c:QF * (c + 1)],
            w_qkv[D + HD * c:D + HD * (c + 1)],
            w_qkv[D + HD * NCORES + HD * c:D + HD * NCORES + HD * (c + 1)],
        ], axis=0)                                      # [768, 4096]
        wq_c = np.ascontiguousarray(rows.T).astype(BF_NP)             # [4096, 768]
        wo_c = np.ascontiguousarray(w_o[:, QF * c:QF * (c + 1)].T).astype(BF_NP)
        in_maps.append({"hT": hT, "wq": wq_c, "wo": wo_c,
                        "cc": ccm, "ss": ssm, "mk": mkm})
    return in_maps


def run(positions, hidden_states, w_qkv, w_o, trace=False):
    nc = _get_nc()
    in_maps = _in_maps(positions, hidden_states, w_qkv, w_o)
    res = run_bass_kernel_spmd(nc, in_maps, list(range(NCORES)), trace=trace)
    parts = np.stack([np.asarray(res.results[i]["out"], dtype=np.float64)
                      for i in range(NCORES)], axis=0)
    full = parts.sum(axis=0).astype(np.float32)
    return full, res


def kernel(positions, hidden_states, w_qkv, w_o):
    full, _ = run(positions, hidden_states, w_qkv, w_o, trace=False)
    return full


# revision 9
# speedup vs baseline: 1.1189x; 1.0441x over previous
"""Grok1-style attention on 8 trn2 NeuronCores, tensor-parallel over heads.

Sharding (per core c of 8):
  - q heads 4c..4c+3 (512 features), kv head c (128+128 features)
  - w_qkv sharded column-wise (by head), w_o row-wise; partial o_proj
    outputs summed on host (the all-reduce).

Device layout: qkv computed TRANSPOSED (features on partitions, positions
on free axis) so scores (k^T q), probs*V and o_proj chain without
transposes except 16 cheap PE transposes of V.

Perf structure:
  - all inputs host-cast to bf16, loads split across 3 DMA rings
    (gpsimd / sync / scalar) so the first qkv matmul starts early.
  - PSUM used as four 2-bank PAIR tiles: "sc" ring x2 (scores/o_proj/
    qkv) + "acc" ring x2 ([attn | denom] per head).  One tanh and one
    exp instruction covers a 2-bank score pair -> half the ACT
    instruction overhead, and the ACT->PE chain has a 2-pair runway.
  - causal mask is ADDITIVE (-3000 pre-DMAed into the PSUM bank before
    the score matmul accumulates onto it with start=False): tanh
    saturates to -1, exp gives e^-30 ~ 1e-13 -> no per-tile mask
    multiply in the critical chain at all.
  - denominator rows via an all-ones [128,128] lhsT (broadcast row-sum
    on all PSUM partitions), reciprocal_approx_fast (~0.65us, 18-bit)
    instead of the 3.3us exact reciprocal.
  - o_proj(qt) interleaved after each qt's heads; bf16 partial outputs
    written as 1024-col pairs; host sums partials in fp64.
"""
import os
import numpy as np
from contextlib import ExitStack

ADD_MASK = os.environ.get("K_ADD_MASK", "1") == "1"
PAIR_ACT = os.environ.get("K_PAIR_ACT", "1") == "1"


import concourse.bass as bass
import concourse.mybir as mybir
import concourse.tile as tile
from concourse import bacc
from concourse.bass_utils import run_bass_kernel_spmd
from concourse.masks import make_identity

T = 2048
D = 4096
HD = 128
HALF = 64
NCORES = 8
HPC = 4                    # q heads per core
QF = HPC * HD              # 512
NF = QF + 2 * HD           # 768 qkv features per core
NCH = D // 128             # 32 contraction chunks
TT = 512                   # t-tile width (matmul moving dim)
NTT = T // TT              # 4
NKT = T // 128             # 16 k-tiles
SCALING = HD ** -0.5
CAP = 30.0
MASKNEG = -3000.0
BF = mybir.dt.bfloat16
F32 = mybir.dt.float32
BF_NP = mybir.dt.np(BF)


def _emit(nc):
    hT = nc.dram_tensor("hT", [D, T], BF, kind="ExternalInput").ap()
    wq = nc.dram_tensor("wq", [D, NF], BF, kind="ExternalInput").ap()
    wo = nc.dram_tensor("wo", [QF, D], BF, kind="ExternalInput").ap()
    cc = nc.dram_tensor("cc", [HD, T], BF, kind="ExternalInput").ap()
    ss = nc.dram_tensor("ss", [HD, T], BF, kind="ExternalInput").ap()
    mg = nc.dram_tensor("mg", [4, 128, TT], BF, kind="ExternalInput").ap()
    out = nc.dram_tensor("out", [T, D], BF, kind="ExternalOutput").ap()

    with tile.TileContext(nc) as tc:
        with ExitStack() as ctx:
            wqp = ctx.enter_context(tc.tile_pool(name="wqp", bufs=1))
            hp = ctx.enter_context(tc.tile_pool(name="hp", bufs=4))
            cstp = ctx.enter_context(tc.tile_pool(name="cstp", bufs=1))
            seqp = ctx.enter_context(tc.tile_pool(name="seqp", bufs=1))
            rtp = ctx.enter_context(tc.tile_pool(name="rtp", bufs=2))
            stp = ctx.enter_context(tc.tile_pool(name="stp", bufs=2))
            etp = ctx.enter_context(tc.tile_pool(name="etp", bufs=3))
            smp = ctx.enter_context(tc.tile_pool(name="smp", bufs=2))
            obp = ctx.enter_context(tc.tile_pool(name="obp", bufs=3))
            psp = ctx.enter_context(tc.tile_pool(name="psp", bufs=2, space="PSUM"))

            # ---- resident loads, split across the 3 DMA rings ----
            wq_r = wq.rearrange("(c p) f -> p c f", p=128)
            wq_t = []
            for g in range(8):
                w_g = wqp.tile([128, 4, NF], BF, tag=f"wq{g}", name=f"wq{g}")
                nc.gpsimd.dma_start(w_g[:], wq_r[:, 4 * g:4 * g + 4, :])
                wq_t.append(w_g)
            cc_sb = cstp.tile([HD, T], BF, tag="cc")
            ss_sb = cstp.tile([HD, T], BF, tag="ss")
            nc.scalar.dma_start(cc_sb[:], cc[:, :])
            nc.scalar.dma_start(ss_sb[:], ss[:, :])
            mneg = cstp.tile([128, 4, TT], BF, tag="mneg")
            nc.scalar.dma_start(mneg[:], mg.rearrange("m p t -> p m t"))
            wo_r = wo.rearrange("(c p) n -> p c n", p=128)
            wo_t = []
            for j in range(2):
                w_j = wqp.tile([128, 2, D], BF, tag=f"wo{j}", name=f"wo{j}")
                nc.gpsimd.dma_start(w_j[:], wo_r[:, 2 * j:2 * j + 2, :])
                wo_t.append(w_j)
            ident = cstp.tile([128, 128], BF, tag="id")
            make_identity(nc, ident[:])
            ones_kk = cstp.tile([128, 128], BF, tag="ones_kk")
            nc.gpsimd.memset(ones_kk[:], 1.0)

            qTt = [[seqp.tile([HD, TT], BF, tag=f"q{h}_{tt}", name=f"qT{h}_{tt}")
                    for tt in range(NTT)] for h in range(HPC)]
            kTt = [seqp.tile([HD, TT], BF, tag=f"k_{tt}", name=f"kT{tt}")
                   for tt in range(NTT)]
            vTt = [seqp.tile([HD, TT], BF, tag=f"v_{tt}", name=f"vT{tt}")
                   for tt in range(NTT)]
            vbt = [seqp.tile([128, HD], BF, tag=f"vb_{kt}", name=f"vb{kt}")
                   for kt in range(NKT)]
            atq = [[seqp.tile([HD, TT], BF, tag=f"a{h}_{qt}", name=f"at{h}_{qt}")
                    for qt in range(NTT)] for h in range(HPC)]

            # ---- phase 1: qkv projection (transposed) + rope ----
            hT_r = hT.rearrange("(c p) t -> p c t", p=128)
            for tt in range(NTT):
                t0 = tt * TT
                h_t = []
                for i in range(4):
                    h_i = hp.tile([128, 8, TT], BF, tag="h", name=f"h{tt}_{i}")
                    eng = nc.sync if i % 2 == 0 else nc.scalar
                    eng.dma_start(h_i[:], hT_r[:, 8 * i:8 * i + 8, t0:t0 + TT])
                    h_t.append(h_i)
                c_t = cc_sb[:, t0:t0 + TT]
                s_t = ss_sb[:, t0:t0 + TT]
                for fg in range(2):
                    pA = psp.tile([128, 2, TT], F32, tag="sc", name="qkv_A")
                    pB = psp.tile([128, 2, TT], F32, tag="sc", name="qkv_B")
                    tgt = [pA[:, 0, :], pA[:, 1, :], pB[:, 0, :]]
                    for c in range(NCH):
                        src = h_t[c // 8][:, c % 8, :]
                        for j in range(3):
                            f = fg * 3 + j
                            nc.tensor.matmul(
                                tgt[j],
                                wq_t[c // 4][:, c % 4, f * 128:(f + 1) * 128],
                                src,
                                start=(c == 0),
                                stop=(c == NCH - 1),
                            )
                    for j in range(3):
                        f = fg * 3 + j
                        if f < 5:
                            dst = qTt[f][tt] if f < HPC else kTt[tt]
                            qk_sb = rtp.tile([128, TT], BF, tag="qk_sb")
                            nc.scalar.copy(qk_sb[:], tgt[j])
                            # rotated copy: [x2; x1] via partition-swap DMA
                            rot = rtp.tile([128, TT], BF, tag="rot")
                            nc.gpsimd.dma_start(rot[0:HALF, :], qk_sb[HALF:128, :])
                            nc.gpsimd.dma_start(rot[HALF:128, :], qk_sb[0:HALF, :])
                            m1 = rtp.tile([128, TT], BF, tag="m1")
                            nc.vector.tensor_mul(m1[:], qk_sb[:], c_t)
                            m2 = rtp.tile([128, TT], BF, tag="m2")
                            nc.vector.tensor_mul(m2[:], rot[:], s_t)
                            nc.vector.tensor_add(dst[:], m1[:], m2[:])
                        else:
                            nc.scalar.copy(vTt[tt][:], tgt[j])
                # transpose this t-tile's V to [t, d] blocks (PSUM bf16)
                tp = psp.tile([128, 4, 128], BF, tag="sc", name="tp")
                for i in range(4):
                    kt = 4 * tt + i
                    nc.tensor.transpose(
                        tp[:, i, :],
                        vTt[tt][:, i * 128:(i + 1) * 128],
                        ident[:])
                    nc.vector.tensor_copy(vbt[kt][:], tp[:, i, :])

            # ---- phase 2+3: attention (paired banks, additive mask,
            #      2-pair runway) + o_proj per qt ----
            def emit_score_pair(qt, h, p):
                sp = psp.tile([128, 2, TT], F32, tag="sc", name="s_pair")
                info = []
                for s_i in range(2):
                    kt = 2 * p + s_i
                    m = kt - 4 * qt
                    j0 = 128 * m if m >= 0 else 0
                    if ADD_MASK and m >= 0:
                        nc.tensor.matmul(
                            sp[:, s_i, j0:TT], ident[:], mneg[:, m, j0:TT],
                            start=True, stop=False,
                        )
                    nc.tensor.matmul(
                        sp[:, s_i, j0:TT],
                        kTt[kt // 4][:, (kt % 4) * 128:(kt % 4 + 1) * 128],
                        qTt[h][qt][:, j0:TT],
                        start=(not ADD_MASK) or m < 0, stop=True,
                    )
                    info.append((kt, j0))
                return sp, info

            for qt in range(NTT):
                for h in range(HPC):
                    acc = psp.tile([HD, 2, TT], F32, tag="acc", name="acc")
                    nkt = 4 * qt + 4
                    NP = nkt // 2
                    pend = [emit_score_pair(qt, h, p) for p in range(min(2, NP))]
                    for p in range(NP):
                        sp, info = pend[p]
                        st = stp.tile([128, 2, TT], F32, tag="st")
                        et = etp.tile([128, 2, TT], BF, tag="et")
                        if PAIR_ACT:
                            nc.scalar.activation(
                                st[:, :, :], sp[:, :, :],
                                mybir.ActivationFunctionType.Tanh,
                                scale=SCALING / CAP,
                            )
                            nc.scalar.activation(
                                et[:, :, :], st[:, :, :],
                                mybir.ActivationFunctionType.Exp,
                                scale=CAP,
                            )
                        else:
                            for s_i, (kt, j0) in enumerate(info):
                                nc.scalar.activation(
                                    st[:, s_i, j0:TT], sp[:, s_i, j0:TT],
                                    mybir.ActivationFunctionType.Tanh,
                                    scale=SCALING / CAP,
                                )
                                nc.scalar.activation(
                                    et[:, s_i, j0:TT], st[:, s_i, j0:TT],
                                    mybir.ActivationFunctionType.Exp,
                                    scale=CAP,
                                )
                        for s_i, (kt, j0) in enumerate(info):
                            m = kt - 4 * qt
                            if not ADD_MASK and m >= 0:
                                nc.gpsimd.tensor_mul(
                                    et[:, s_i, j0:TT], et[:, s_i, j0:TT],
                                    mneg[:, m, j0:TT])
                            last = kt == nkt - 1
                            nc.tensor.matmul(
                                acc[:, 0, j0:TT], vbt[kt][:], et[:, s_i, j0:TT],
                                start=(kt == 0), stop=last,
                            )
                            nc.tensor.matmul(
                                acc[:, 1, j0:TT], ones_kk[:], et[:, s_i, j0:TT],
                                start=(kt == 0), stop=last,
                            )
                        if p + 2 < NP:
                            pend.append(emit_score_pair(qt, h, p + 2))
                    rcp = smp.tile([128, TT], F32, tag="rcp")
                    nc.vector.reciprocal_approx_fast(rcp[:], acc[:, 1, :])
                    nc.vector.tensor_mul(atq[h][qt][:], acc[:, 0, :], rcp[:])

                # o_proj for this qt's 512 rows, paired output banks
                for t16 in range(4 * qt, 4 * qt + 4):
                    r0 = t16 * 128
                    for half in range(2):
                        for np_ in range(2):
                            n0 = (half * 4 + np_ * 2) * TT
                            pl = psp.tile([128, 2, TT], F32, tag="sc",
                                          name="o_pair")
                            for sub in range(2):
                                for fc in range(HPC):
                                    lhsT = atq[fc][qt][:, (t16 % 4) * 128:
                                                       (t16 % 4 + 1) * 128]
                                    nc.tensor.matmul(
                                        pl[:, sub, :], lhsT,
                                        wo_t[fc // 2][:, fc % 2,
                                                      n0 + sub * TT:n0 + (sub + 1) * TT],
                                        start=(fc == 0), stop=(fc == HPC - 1),
                                    )
                            ob = obp.tile([128, 2, TT], BF, tag="ob")
                            nc.vector.tensor_copy(ob[:], pl[:])
                            nc.sync.dma_start(
                                out[r0:r0 + 128, n0:n0 + 2 * TT],
                                ob[:].rearrange("p a t -> p (a t)"))
    return nc


_CACHE = {}


def _get_nc():
    if "nc" not in _CACHE:
        nc = bacc.Bacc("TRN2", target_bir_lowering=False, debug=False)
        _emit(nc)
        nc.compile()
        _CACHE["nc"] = nc
    return _CACHE["nc"]


def _in_maps(positions, hidden_states, w_qkv, w_o):
    hidden_states = np.asarray(hidden_states, dtype=np.float32)
    w_qkv = np.asarray(w_qkv, dtype=np.float32)
    w_o = np.asarray(w_o, dtype=np.float32)
    pos = np.asarray(positions).astype(np.float64)

    hT = np.ascontiguousarray(hidden_states.T).astype(BF_NP)
    inv_freq = 1.0 / (10000.0 ** (np.arange(HALF, dtype=np.float64) * 2.0 / HD))
    ang = np.outer(inv_freq, pos)                      # [64, T]
    cos = np.cos(ang).astype(np.float32)
    sin = np.sin(ang).astype(np.float32)
    ccm = np.concatenate([cos, cos], axis=0).astype(BF_NP)   # [128, T]
    ssm = np.concatenate([-sin, sin], axis=0).astype(BF_NP)  # [128, T]
    ii = np.arange(128)[:, None]
    jj = np.arange(TT)[None, :]
    mgm = np.stack([(jj - ii - 128 * m < 0) for m in range(4)])
    mgm = (mgm * MASKNEG).astype(BF_NP)                # 0 where keep, -3000 mask

    in_maps = []
    for c in range(NCORES):
        rows = np.concatenate([
            w_qkv[QF * c:QF * (c + 1)],
            w_qkv[D + HD * c:D + HD * (c + 1)],
            w_qkv[D + HD * NCORES + HD * c:D + HD * NCORES + HD * (c + 1)],
        ], axis=0)                                      # [768, 4096]
        wq_c = np.ascontiguousarray(rows.T).astype(BF_NP)             # [4096, 768]
        wo_c = np.ascontiguousarray(w_o[:, QF * c:QF * (c + 1)].T).astype(BF_NP)
        in_maps.append({"hT": hT, "wq": wq_c, "wo": wo_c,
                        "cc": ccm, "ss": ssm, "mg": mgm})
    return in_maps


def run(positions, hidden_states, w_qkv, w_o, trace=False):
    nc = _get_nc()
    in_maps = _in_maps(positions, hidden_states, w_qkv, w_o)
    res = run_bass_kernel_spmd(nc, in_maps, list(range(NCORES)), trace=trace)
    parts = np.stack([np.asarray(res.results[i]["out"], dtype=np.float64)
                      for i in range(NCORES)], axis=0)
    full = parts.sum(axis=0).astype(np.float32)
    return full, res


def kernel(positions, hidden_states, w_qkv, w_o):
    full, _ = run(positions, hidden_states, w_qkv, w_o, trace=False)
    return full
